# revision 53
# baseline (speedup 1.0000x reference)
"""EvolveGCNO RecurrentGCN forward on 8 trn2 NeuronCores.

Strategy (dst-sharded gather, v4):
  - Nodes sharded by destination across 8 cores (6250 each, padded to 6272).
    Edges live on the core owning their dst. Self-loops are NOT materialized
    as edges: their contribution (xs_own rows) is added with one identity-rhs
    PE matmul per 128-dst window, straight from SBUF.
  - Phase A (device): per-core degree via padded-CSR row sums (slot 0 holds
    the self-loop weight 1.0), dinv = 1/sqrt(deg); scale own x rows by dinv
    -> xs (f16) and AllGather the halves so every core holds the full scaled
    feature table. Gathered rows carry dinv[src]; dinv[dst] scaling is
    deferred to the very end (valid because dinv >= 0 commutes with relu and
    the channel-mixing matmuls), applied to the final [1, nd] row.
  - Phase B (device): per block of 512 dsts, dma_gather xs[src] rows (256 B
    f16 descriptors - the memory-roofline term; the cost model prices any
    descriptor < 512 B at the same 1.42 ns, so f16 matches f32 cost while
    halving SBUF/collective bytes) from the two int16-addressable halves of
    the table. Node placement is degree-balanced (in-degree LPT across
    windows; greedy lo/hi split of each node's out-edges via its row class),
    so edges pack into K0=6 base columns per (window, half) plus one
    512-wide per-block overflow column (applied as per-window 128-wide
    segments), minimizing gather descriptors (~78k/core vs ~100k unbalanced). One-hot scatter matrices
    S[e, j] = (iota == off) * ew (f16) are built a block ahead on the DVE
    (software-pipelined like the gathers, keeping the PE queue deep and the
    PE at full p-state); PE matmuls (f16, 1 cycle/row) aggregate into PSUM
    [128f, BLK d]; evolved W (f16), ReLU, lin_w follow; the final row is
    scaled by dinv[dst] and written out.
  - GRU weight evolution on device (replicated on every core, f32).

Host work is limited to graph partitioning / index manipulation / layout
(sorting, bincount, padding, parameter transposes, staging edge weights into
their device dtypes); the reference computation's floating point math
(degrees, rsqrt, scaling, aggregation, GRU, linear) happens on device.
"""

import math
import sys

import numpy as np

sys.path.insert(0, "/opt/trn_rl_repo")

N_NODES, N_EDGES, C = 50000, 600000, 128
NCORES = 8
NPC = N_NODES // NCORES            # 6250 nodes per core
NTILE = (NPC + 127) // 128         # 49 sbuf tiles of 128 nodes
NPAD = NTILE * 128                 # 6272 padded nodes per core
HALFL = NPAD // 2                  # 3136: per-core split row for the tables
HALF = NCORES * HALFL              # 25088 rows per gather table (int16-safe)
WDST = 128                         # dsts per psum column window
NWINDOW = NPAD // WDST             # 49 windows per core
BLK = 512                          # dsts per psum block
WPB = BLK // WDST                  # full-block windows (4)
NBLK = (NPAD + BLK - 1) // BLK     # 13 blocks (last block: 1 window)


# ---------------------------------------------------------------------------
# Host-side preprocessing: graph partitioning + layout (index work only)
# ---------------------------------------------------------------------------

def _balance_layout(src_a, dst_a):
    """Node id -> (core, local slot l) placement. Cores keep their
    contiguous id range; within a core, windows (l // 128) are LPT-packed
    by in-degree so every (window) receives ~equal edge counts, and the
    row class (l % 128 < 64 -> table half A) is chosen greedily to balance
    each destination window's lo/hi split. Pure index work."""
    indeg = np.bincount(dst_a, minlength=N_NODES)
    l_of = np.zeros(N_NODES, np.int64)
    for c in range(NCORES):
        ids = np.arange(c * NPC, (c + 1) * NPC)
        deg = indeg[ids]
        order = np.argsort(-deg, kind="stable")
        wcnt = np.zeros(NWINDOW, np.int64)
        wload = np.zeros(NWINDOW, np.float64)
        cap = np.full(NWINDOW, 128, np.int64)
        cap[NWINDOW - 1] = NPC - 128 * (NWINDOW - 1)
        win_of = np.zeros(NPC, np.int64)
        # LPT: heaviest first into the lightest non-full window
        for i in order:
            wl = np.where(wcnt < cap, wload, np.inf)
            w = int(np.argmin(wl))
            win_of[i] = w
            wcnt[w] += 1
            wload[w] += deg[i]
        l_of[ids] = win_of  # temporarily store window; rows fixed later
    # greedy half assignment balancing each (dst window, half) count
    gwin = (dst_a // NPC) * NWINDOW + l_of[dst_a]    # global window per edge
    nG = NCORES * NWINDOW
    cnt_h = np.zeros((nG, 2), np.int64)
    # out-edges grouped by source node
    o = np.argsort(src_a, kind="stable")
    so, go = src_a[o], gwin[o]
    starts = np.searchsorted(so, np.arange(N_NODES))
    ends = np.searchsorted(so, np.arange(N_NODES) + 1)
    half_of = np.zeros(N_NODES, np.int8)
    acap = np.zeros((NCORES, NWINDOW), np.int64)
    bcap = np.zeros((NCORES, NWINDOW), np.int64)
    for c in range(NCORES):
        ids = np.arange(c * NPC, (c + 1) * NPC)
        wcnt = np.bincount(l_of[ids], minlength=NWINDOW)
        acap[c] = np.minimum(64, wcnt)
        bcap[c] = wcnt - acap[c]
    odeg = ends - starts
    for n in np.argsort(-odeg, kind="stable"):
        c, w = n // NPC, l_of[n]
        g = go[starts[n]:ends[n]]
        la = cnt_h[g, 0].sum()
        lb = cnt_h[g, 1].sum()
        h = 0 if (la <= lb or bcap[c, w] == 0) and acap[c, w] > 0 else 1
        half_of[n] = h
        if len(g):
            np.add.at(cnt_h, (g, h), 1)
        if h == 0:
            acap[c, w] -= 1
        else:
            bcap[c, w] -= 1
    # assign concrete rows: class A -> rows 0..63, class B -> 64..127
    for c in range(NCORES):
        ids = np.arange(c * NPC, (c + 1) * NPC)
        winsnap = l_of[ids].copy()
        for w in range(NWINDOW):
            m = ids[winsnap == w]
            ha = m[half_of[m] == 0]
            hb = m[half_of[m] == 1]
            rows = np.concatenate([np.arange(len(ha)),
                                   64 + np.arange(len(hb))])
            l_of[np.concatenate([ha, hb])] = w * 128 + rows
    return l_of


def preprocess(edge_index: np.ndarray, edge_weight: np.ndarray):
    src_a = np.asarray(edge_index[0], dtype=np.int64)
    dst_a = np.asarray(edge_index[1], dtype=np.int64)
    ew_a = np.asarray(edge_weight, dtype=np.float32)

    l_of = _balance_layout(src_a, dst_a)
    # remap node ids to their placed slot: node -> core * NPC + l
    place = (np.arange(N_NODES) // NPC) * NPC + l_of
    src_a = place[src_a]
    dst_a = place[dst_a]

    core_of = dst_a // NPC
    percore = []
    kdeg = 2
    gcnts = np.zeros((NCORES, 2 * NWINDOW), np.int64)
    for c in range(NCORES):
        m = core_of == c
        s = src_a[m]
        l = (dst_a[m] - c * NPC).astype(np.int64)
        w = ew_a[m]
        # gather-table row of src s (r-major per-core layout, see xs write)
        co = s // NPC
        ls = s % NPC
        pi = (ls % 128) * NTILE + (ls // 128)
        half = (pi >= HALFL).astype(np.int64)
        idx16 = (co * HALFL + (pi - half * HALFL)).astype(np.int64)
        # group edges by (window, half), dst-sorted within each group
        key = (l // WDST) * 2 + half
        order = np.argsort(key * (NPC + 1) + l, kind="stable")
        l, w, half, idx16 = l[order], w[order], half[order], idx16[order]
        win = l // WDST
        percore.append((l, w, half, idx16, win))
        gcnts[c] = np.bincount(win * 2 + half, minlength=2 * NWINDOW)
        kdeg = max(kdeg, int(np.bincount(l, minlength=NPAD).max()) + 1)
    KDEG = kdeg

    nw_b = [min(WPB, NWINDOW - WPB * b) for b in range(NBLK)]
    win_of_g = np.arange(2 * NWINDOW) // 2
    blk_of_g = win_of_g // WPB
    half_of_g = np.arange(2 * NWINDOW) % 2

    def layout(K0):
        # overflow counts per (core, block, half)
        ovf = np.maximum(gcnts - 128 * K0, 0)          # [core, group]
        ov = np.zeros((NCORES, NBLK, 2), np.int64)
        for bb in range(NBLK):
            for h in (0, 1):
                gm = (blk_of_g == bb) & (half_of_g == h)
                ov[:, bb, h] = ovf[:, gm].sum(1)
        ovl_b = [int(np.ceil(ov[:, bb, 0].max() / 128)) for bb in range(NBLK)]
        ovh_b = [int(np.ceil(ov[:, bb, 1].max() / 128)) for bb in range(NBLK)]
        ncols = [nw_b[bb] * 2 * K0 + ovl_b[bb] + ovh_b[bb]
                 for bb in range(NBLK)]
        return ovl_b, ovh_b, sum(ncols)

    # K0=6 balances gather descriptors (the DMA roofline) against the
    # 4x-wide PE/DVE cost of 512-wide overflow columns (measured optimum)
    K0 = 6
    ovl_b, ovh_b, TOT = layout(K0)
    blk_start = np.cumsum(
        [0] + [nw_b[bb] * 2 * K0 + ovl_b[bb] + ovh_b[bb]
               for bb in range(NBLK)])

    metas = []
    for c in range(NCORES):
        l, w, half, idx16, win = percore[c]
        wb = win % WPB                              # window within block
        b = win // WPB
        nw = np.array(nw_b)[b]
        novl = np.array(ovl_b)[b]
        # position within the (window, half) group
        grp = win * 2 + half
        gcnt = gcnts[c]
        gstart = np.cumsum(gcnt) - gcnt
        p_in = np.arange(len(l)) - gstart[grp]
        isbase = p_in < 128 * K0
        j = p_in // 128
        row = p_in % 128
        # overflow edges: running position within the (block, half) ovf list
        ovf_idx = np.zeros(len(l), np.int64)
        ocnt = np.zeros((NBLK, 2), np.int64)
        om = ~isbase
        okey = b * 2 + half
        for bb in range(NBLK):
            for h in (0, 1):
                mm = om & (okey == bb * 2 + h)
                ovf_idx[mm] = np.arange(mm.sum())
                ocnt[bb, h] = mm.sum()
        oj = ovf_idx // 128
        orow_ = ovf_idx % 128

        col = np.where(
            isbase,
            np.where(half == 0,
                     blk_start[b] + wb * K0 + j,
                     blk_start[b] + nw * K0 + novl + wb * K0 + j),
            np.where(half == 0,
                     blk_start[b] + nw * K0 + oj,
                     blk_start[b] + 2 * nw * K0 + novl + oj),
        )
        rr = np.where(isbase, row, orow_)

        offv = np.zeros((128, TOT), np.float32)
        ewv = np.zeros((128, TOT), np.float32)
        offv[rr, col] = np.where(isbase, l % WDST, l % BLK).astype(np.float32)
        ewv[rr, col] = w

        # gather index lists, one per (block, half), packed along columns.
        # list position i = (col_rel * 128 + row); idx 0 pads (killed by ew=0)
        lo_starts, hi_starts = [], []
        lo_c = hi_c = 0
        for bb in range(NBLK):
            nwb = nw_b[bb]
            lo_starts.append(lo_c)
            hi_starts.append(hi_c)
            lo_c += (nwb * K0 + ovl_b[bb]) * 8
            hi_c += (nwb * K0 + ovh_b[bb]) * 8
        CL, CH = lo_c, hi_c
        idxlo = np.zeros((16, CL), np.int16)
        idxhi = np.zeros((16, CH), np.int16)
        for h, (arr, starts) in enumerate(
                [(idxlo, lo_starts), (idxhi, hi_starts)]):
            mh = half == h
            col_rel = np.where(isbase[mh], wb[mh] * K0 + j[mh],
                               nw[mh] * K0 + oj[mh])
            i_list = col_rel * 128 + rr[mh]         # position in block's list
            ci = np.array(starts)[b[mh]] * 16 + i_list  # global flat position
            arr[ci % 16, ci // 16] = idx16[mh]
        metas.append(dict(
            offv=offv, ewv=ewv,
            idxlo=np.ascontiguousarray(np.tile(idxlo, (8, 1))),
            idxhi=np.ascontiguousarray(np.tile(idxhi, (8, 1))),
        ))

    # padded CSR of edge weights for the degree computation; slot 0 is the
    # self-loop weight 1.0 (all rows, incl. pads so deg > 0 everywhere)
    for c in range(NCORES):
        l, w, half, idx16, win = percore[c]
        counts = np.bincount(l, minlength=NPAD)
        starts = np.cumsum(counts) - counts
        o2 = np.argsort(l, kind="stable")
        ls_, ws_ = l[o2], w[o2]
        slot = 1 + np.arange(len(ls_)) - starts[ls_]
        csr = np.zeros((NPAD, KDEG), np.float16)
        csr[:, 0] = 1.0
        csr[ls_, slot] = ws_.astype(np.float16)
        # r-major fat layout: one contiguous span per sbuf partition
        csr_r = np.ascontiguousarray(
            csr.reshape(NTILE, 128, KDEG).transpose(1, 0, 2)
               .reshape(128, NTILE * KDEG))
        metas[c]["csr"] = csr_r

    node_of_slot = np.empty(N_NODES, np.int64)
    node_of_slot[place] = np.arange(N_NODES)
    pre = dict(K0=K0, KDEG=KDEG, TOT=TOT, nw_b=nw_b,
               node_of_slot=node_of_slot,
               ovl_b=ovl_b, ovh_b=ovh_b,
               blk_start=[int(v) for v in blk_start],
               lo_starts=lo_starts, hi_starts=hi_starts, CL=CL, CH=CH)
    return pre, metas


def make_in_maps(inp: dict, pre, metas):
    iota = np.tile(np.arange(WDST, dtype=np.float16), (128, 1))
    W0 = np.asarray(inp["W0"], np.float32)
    x = np.ascontiguousarray(np.asarray(inp["x"], np.float32))
    iota5 = np.tile(np.arange(BLK, dtype=np.float16), (128, 1))
    shared = dict(
        iota=np.ascontiguousarray(iota),
        iota5=np.ascontiguousarray(iota5),
        eye=np.ascontiguousarray(np.eye(128, dtype=np.float16)),
        w0=W0,
        w0t=np.ascontiguousarray(W0.T),
        wiht=np.ascontiguousarray(np.asarray(inp["gru_w_ih"], np.float32).T),
        whht=np.ascontiguousarray(np.asarray(inp["gru_w_hh"], np.float32).T),
        bih=np.asarray(inp["gru_b_ih"], np.float32),
        bhh=np.asarray(inp["gru_b_hh"], np.float32),
        linw=np.ascontiguousarray(np.asarray(inp["lin_w"], np.float32).T),
        linb=np.asarray(inp["lin_b"], np.float32).reshape(1, 1),
    )
    maps = []
    nos = pre["node_of_slot"]
    for c in range(NCORES):
        xo = np.zeros((NPAD, C), np.float32)
        xo[:NPC] = x[nos[c * NPC:(c + 1) * NPC]]
        maps.append(dict(shared, x_own=xo, **metas[c]))
    return maps


# ---------------------------------------------------------------------------
# Device program
# ---------------------------------------------------------------------------

def build_program(pre, debug_taps: bool = False, skip_gather: bool = False,
                  skip_collective: bool = False, nblk: int = NBLK,
                  rep: int = 1, skip_compute: bool = False,
                  gather_elem: int = C, nqueues: int = 4,
                  table_fp8: bool = False, dma_scratch: int = 16384):
    import concourse.bacc as bacc
    import concourse.bass as bass
    import concourse.tile as tile
    from concourse import mybir

    f32 = mybir.dt.float32
    f16 = mybir.dt.float16
    tdt = mybir.dt.float8e3 if table_fp8 else f16
    i16 = mybir.dt.int16
    AF = mybir.ActivationFunctionType
    OP = mybir.AluOpType
    K0, KDEG, TOT = pre["K0"], pre["KDEG"], pre["TOT"]
    nw_b = pre["nw_b"]
    ovl_b, ovh_b = pre["ovl_b"], pre["ovh_b"]
    blk_start = pre["blk_start"]
    lo_starts, hi_starts = pre["lo_starts"], pre["hi_starts"]
    CL, CH = pre["CL"], pre["CH"]
    ncol_b = [nw_b[bb] * 2 * K0 + ovl_b[bb] + ovh_b[bb]
              for bb in range(NBLK)]
    MAXCOL = max(ncol_b)              # widest block in meta columns
    NBASE = WPB * 2 * K0              # base columns in a full block
    OVMAX = max(1, max(ovl_b[bb] + ovh_b[bb] for bb in range(NBLK)))

    nc = bacc.Bacc("TRN2", target_bir_lowering=False, debug=False,
                   num_devices=NCORES, num_swdge_queues=nqueues,
                   dynamic_dma_scratch_size=dma_scratch)

    x_own_t = nc.declare_dram_parameter("x_own", [NPAD, C], f32, isOutput=False)
    idxlo_t = nc.declare_dram_parameter("idxlo", [128, CL], i16, isOutput=False)
    idxhi_t = nc.declare_dram_parameter("idxhi", [128, CH], i16, isOutput=False)
    offv_t = nc.declare_dram_parameter("offv", [128, TOT], f32, isOutput=False)
    ewv_t = nc.declare_dram_parameter("ewv", [128, TOT], f32, isOutput=False)
    csr_t = nc.declare_dram_parameter("csr", [128, NTILE * KDEG], f16,
                                      isOutput=False)
    iota_t = nc.declare_dram_parameter("iota", [128, WDST], f16, isOutput=False)
    iota5_t = nc.declare_dram_parameter("iota5", [128, BLK], f16,
                                        isOutput=False)
    eye_t = nc.declare_dram_parameter("eye", [128, 128], f16, isOutput=False)
    w0_t = nc.declare_dram_parameter("w0", [C, C], f32, isOutput=False)
    w0t_t = nc.declare_dram_parameter("w0t", [C, C], f32, isOutput=False)
    wiht_t = nc.declare_dram_parameter("wiht", [C, 3 * C], f32, isOutput=False)
    whht_t = nc.declare_dram_parameter("whht", [C, 3 * C], f32, isOutput=False)
    bih_t = nc.declare_dram_parameter("bih", [3 * C], f32, isOutput=False)
    bhh_t = nc.declare_dram_parameter("bhh", [3 * C], f32, isOutput=False)
    linw_t = nc.declare_dram_parameter("linw", [C, 1], f32, isOutput=False)
    linb_t = nc.declare_dram_parameter("linb", [1, 1], f32, isOutput=False)
    out_t = nc.declare_dram_parameter("out", [NPAD], f32, isOutput=True)

    xs_own_hbm = nc.dram_tensor("xs_own_hbm", [NPAD, C], tdt)
    dinvrow_hbm = nc.dram_tensor("dinvrow_hbm", [NPAD], f32)
    xs_allA = nc.dram_tensor("xs_allA", [HALF, C], tdt, addr_space="Shared")
    xs_allB = nc.dram_tensor("xs_allB", [HALF, C], tdt, addr_space="Shared")

    def bcast_partitions(ap, parts=128):
        return bass.AP(tensor=ap.tensor, offset=ap.offset,
                       ap=[[0, parts]] + list(ap.ap))

    with tile.TileContext(nc) as tc:
        with (
            tc.tile_pool(name="singles", bufs=1) as singles,
            tc.tile_pool(name="gru", bufs=1) as gru,
            tc.tile_pool(name="gpool", bufs=3) as gpool,
            tc.tile_pool(name="spool", bufs=3) as spool,
            tc.tile_pool(name="sovp", bufs=2) as sovp,
            tc.tile_pool(name="bpool", bufs=3) as bpool,
            tc.tile_pool(name="opool", bufs=2) as opool,
            tc.tile_pool(name="pagg", bufs=2, space="PSUM") as pagg,
            tc.tile_pool(name="ph", bufs=2, space="PSUM") as ph,
            tc.tile_pool(name="po", bufs=2, space="PSUM") as po,
            tc.tile_pool(name="pjunk", bufs=1, space="PSUM") as pjunk,
        ):
            junk_ps = pjunk.tile([1, 1], f32, tag="junk")

            # zero the gather buffers once (Pool is idle at program start):
            # truncated gathers leave tail slots of the last overflow column
            # unwritten; first use must be finite (0 * ew=0 contributions),
            # later reuses hold stale-but-finite rows from the prior block
            for _gi in range(3):
                _gz = gpool.tile([128, MAXCOL, gather_elem], tdt, tag="g")
                nc.gpsimd.memset(_gz[:], 0.0)

            def pe_absorb(ap):
                nc.tensor.matmul(junk_ps[:1, :1], lhsT=ap, rhs=ap,
                                 start=True, stop=True)

            # ---------------- phase A: deg -> dinv -> xs -> allgather ----
            # critical-path loads first: csr (degree), then x halves
            csr_sb = singles.tile([128, NTILE, KDEG], f16)
            nc.sync.dma_start(
                csr_sb[:], csr_t[:].rearrange("r (t k) -> r t k", t=NTILE))
            eye_sb = singles.tile([128, 128], f16)
            nc.sync.dma_start(eye_sb[:], eye_t[:])
            deg_sb = singles.tile([128, NTILE], f32)
            for t in range(NTILE):
                nc.vector.reduce_sum(deg_sb[:, t:t + 1], csr_sb[:, t, :],
                                     axis=mybir.AxisListType.X)
            sqrt_sb = singles.tile([128, NTILE], f32)
            nc.scalar.activation(sqrt_sb[:], deg_sb[:], AF.Sqrt)
            dinv_sb = singles.tile([128, NTILE], f32)
            nc.vector.reciprocal(dinv_sb[:], sqrt_sb[:])
            dinv16_sb = singles.tile([128, NTILE], f16)
            nc.scalar.activation(dinv16_sb[:], dinv_sb[:], AF.Copy)
            # transpose dinv to [tile, row] so window slices are [1, 128] rows
            dinvT_ps = pjunk.tile([NTILE, 128], f32, tag="dinvT")
            nc.tensor.matmul(dinvT_ps[:], lhsT=dinv16_sb[:], rhs=eye_sb[:],
                             start=True, stop=True)
            dinvT_sb = singles.tile([NTILE, 128], f32)
            nc.scalar.activation(dinvT_sb[:], dinvT_ps[:], AF.Copy)
            # flatten to a single-partition row (engines need partition-0 APs)
            nc.sync.dma_start(
                dinvrow_hbm[:].rearrange("(t r) -> t r", t=NTILE),
                dinvT_sb[:])


            xown_sb = singles.tile([128, NTILE, C], f32)
            xs_sb = singles.tile([128, NTILE, C], f16)
            if table_fp8:
                xs_tab_sb = singles.tile([128, NTILE, C], tdt)
            else:
                xs_tab_sb = xs_sb
            nc.sync.dma_start(
                xown_sb[:], x_own_t[:].rearrange("(t r) f -> r t f", r=128))

            # the gather index tables must precede the first gather's
            # desc-gen; everything else can stream after the xs writes
            idxlo_sb = singles.tile([128, CL], i16)
            nc.sync.dma_start(idxlo_sb[:], idxlo_t[:])
            idxhi_sb = singles.tile([128, CH], i16)
            nc.sync.dma_start(idxhi_sb[:], idxhi_t[:])

            # produce the table in halves (A = partitions 0:64, B = 64:128)
            # so the A AllGather and block-0 lo-gather start early
            for hp, (p0, p1) in enumerate(((0, 64), (64, 128))):
                for t in range(NTILE):
                    # spread the 2x49 row-scale ops over DVE/ACT/Pool so the
                    # serial chain is ~1/3 as deep (engines run in parallel)
                    r = t % 4
                    if r == 2:
                        nc.scalar.activation(
                            xs_sb[p0:p1, t, :], xown_sb[p0:p1, t, :],
                            AF.Copy, scale=dinv_sb[p0:p1, t:t + 1])
                    else:
                        nc.vector.tensor_scalar(
                            out=xs_sb[p0:p1, t, :], in0=xown_sb[p0:p1, t, :],
                            scalar1=dinv_sb[p0:p1, t:t + 1], scalar2=None,
                            op0=OP.mult)
                if table_fp8:
                    nc.scalar.activation(
                        xs_tab_sb[p0:p1].rearrange("r t f -> r (t f)"),
                        xs_sb[p0:p1].rearrange("r t f -> r (t f)"), AF.Copy)
                # r-major write: one fat contiguous span per partition
                half_hbm = (xs_own_hbm[:HALFL, :] if hp == 0
                            else xs_own_hbm[HALFL:, :])
                nc.sync.dma_start(
                    half_hbm.rearrange("(r t) f -> r t f", r=64),
                    xs_tab_sb[p0:p1])
                xs_all = xs_allA if hp == 0 else xs_allB
                if skip_collective:
                    nc.sync.dma_start(xs_all[:HALFL, :], half_hbm)
                else:
                    nc.gpsimd.collective_compute(
                        "AllGather",
                        OP.bypass,
                        replica_groups=[list(range(NCORES))],
                        ins=[half_hbm.opt()],
                        outs=[xs_all[:].opt()],
                    )

            # deferred metadata loads: interleave into the gather stream
            iota_sb = singles.tile([128, WDST], f16)
            nc.sync.dma_start(iota_sb[:], iota_t[:])
            iota5_sb = singles.tile([128, BLK], f16)
            nc.sync.dma_start(iota5_sb[:], iota5_t[:])
            offv_sb = singles.tile([128, TOT], f32)
            nc.sync.dma_start(offv_sb[:], offv_t[:])
            ewv_sb = singles.tile([128, TOT], f32)
            nc.sync.dma_start(ewv_sb[:], ewv_t[:])
            linw_sb = singles.tile([C, 1], f32)
            nc.sync.dma_start(linw_sb[:], linw_t[:])
            pe_absorb(linw_sb[:1, :1])
            linw16_sb = singles.tile([C, 1], f16)
            nc.scalar.activation(linw16_sb[:], linw_sb[:], AF.Copy)
            linb_sb = singles.tile([1, 1], f32)
            nc.sync.dma_start(linb_sb[:], linb_t[:])

            # ---------------- GRU weight evolution ----------------------
            w0_sb = gru.tile([C, C], f32)
            nc.sync.dma_start(w0_sb[:], w0_t[:])
            w0t_sb = gru.tile([C, C], f32)
            nc.sync.dma_start(w0t_sb[:], w0t_t[:])
            pe_absorb(w0t_sb[:1, :1])
            wiht_sb = gru.tile([C, 3 * C], f32)
            nc.sync.dma_start(wiht_sb[:], wiht_t[:])
            whht_sb = gru.tile([C, 3 * C], f32)
            nc.sync.dma_start(whht_sb[:], whht_t[:])
            bihb_sb = gru.tile([128, 3 * C], f32)
            nc.gpsimd.dma_start(out=bihb_sb[:], in_=bcast_partitions(bih_t[:]))
            bhhb_sb = gru.tile([128, 3 * C], f32)
            nc.gpsimd.dma_start(out=bhhb_sb[:], in_=bcast_partitions(bhh_t[:]))

            gx_ps = pagg.tile([128, 3 * C], f32, tag="agg_ps")
            nc.tensor.matmul(gx_ps[:], lhsT=w0t_sb[:], rhs=wiht_sb[:],
                             start=True, stop=True)
            gxb = gru.tile([128, 3 * C], f32)
            nc.vector.tensor_tensor(out=gxb[:], in0=gx_ps[:], in1=bihb_sb[:],
                                    op=OP.add)
            gh_ps = pagg.tile([128, 3 * C], f32, tag="agg_ps")
            nc.tensor.matmul(gh_ps[:], lhsT=w0t_sb[:], rhs=whht_sb[:],
                             start=True, stop=True)
            ghb = gru.tile([128, 3 * C], f32)
            nc.vector.tensor_tensor(out=ghb[:], in0=gh_ps[:], in1=bhhb_sb[:],
                                    op=OP.add)
            rz_in = gru.tile([128, 2 * C], f32)
            nc.vector.tensor_tensor(out=rz_in[:], in0=gxb[:, :2 * C],
                                    in1=ghb[:, :2 * C], op=OP.add)
            rz = gru.tile([128, 2 * C], f32)
            nc.scalar.activation(rz[:], rz_in[:], AF.Sigmoid)
            t1 = gru.tile([128, C], f32)
            nc.vector.tensor_tensor(out=t1[:], in0=rz[:, :C],
                                    in1=ghb[:, 2 * C:], op=OP.mult)
            t2 = gru.tile([128, C], f32)
            nc.vector.tensor_tensor(out=t2[:], in0=gxb[:, 2 * C:], in1=t1[:],
                                    op=OP.add)
            n_sb = gru.tile([128, C], f32)
            nc.scalar.activation(n_sb[:], t2[:], AF.Tanh)
            d_sb = gru.tile([128, C], f32)
            nc.vector.tensor_tensor(out=d_sb[:], in0=w0_sb[:], in1=n_sb[:],
                                    op=OP.subtract)
            e_sb = gru.tile([128, C], f32)
            nc.vector.tensor_tensor(out=e_sb[:], in0=rz[:, C:], in1=d_sb[:],
                                    op=OP.mult)
            w_sb = gru.tile([C, C], f32)
            nc.vector.tensor_tensor(out=w_sb[:], in0=n_sb[:], in1=e_sb[:],
                                    op=OP.add)
            w16_sb = gru.tile([C, C], f16)
            nc.scalar.activation(w16_sb[:], w_sb[:], AF.Copy)

            # ---------------- phase B: gather + aggregate ----------------
            blk_list = [bb for _ in range(rep) for bb in range(nblk)]
            gtiles = {}
            stiles = {}

            def block_col_map(b):
                # returns (base_cols, ovf_cols): base as (w, h, j, col_abs,
                # sidx), ovf as (col_abs, k) with col_abs block-internal
                nw = nw_b[b]
                novl = ovl_b[b]
                base, ovf = [], []
                for w in range(nw):
                    for h in (0, 1):
                        for j in range(K0):
                            ca = (w * K0 + j if h == 0
                                  else nw * K0 + novl + w * K0 + j)
                            base.append((w, h, j, ca))
                for k in range(novl):
                    ovf.append((nw * K0 + k, k))
                for k in range(ovh_b[b]):
                    ovf.append((2 * nw * K0 + novl + k, novl + k))
                return base, ovf

            def issue_s_builds(bi):
                b = blk_list[bi]
                c0 = blk_start[b]
                base, ovf = block_col_map(b)
                s_blk = spool.tile([128, NBASE, WDST], f16, tag="s")
                sov = sovp.tile([128, OVMAX, BLK], f16, tag="sov")
                if not skip_compute:
                    for bidx, (w, h, j, ca) in enumerate(base):
                        nc.vector.tensor_scalar(
                            out=s_blk[:, bidx, :],
                            in0=iota_sb[:],
                            scalar1=offv_sb[:, c0 + ca:c0 + ca + 1],
                            scalar2=ewv_sb[:, c0 + ca:c0 + ca + 1],
                            op0=OP.is_equal,
                            op1=OP.mult,
                        )
                    for ca, k in ovf:
                        nc.vector.tensor_scalar(
                            out=sov[:, k, :],
                            in0=iota5_sb[:],
                            scalar1=offv_sb[:, c0 + ca:c0 + ca + 1],
                            scalar2=ewv_sb[:, c0 + ca:c0 + ca + 1],
                            op0=OP.is_equal,
                            op1=OP.mult,
                        )
                stiles[bi] = (s_blk, sov)

            def issue_block_loads(bi):
                b = blk_list[bi]
                nw = nw_b[b]
                nlo = nw * K0 + ovl_b[b]
                nhi = nw * K0 + ovh_b[b]
                g_sb = gpool.tile([128, MAXCOL, gather_elem], tdt, tag="g")
                if not skip_gather:
                    nc.gpsimd.dma_gather(
                        g_sb[:, :nlo, :],
                        xs_allA[:, :gather_elem],
                        idxlo_sb[:, lo_starts[b]:lo_starts[b] + nlo * 8],
                        nlo * 128,
                        nlo * 128,
                        gather_elem,
                        elem_step=C,
                        single_packet=False,
                        queue_num=(2 * b) % nqueues,
                    )
                    nc.gpsimd.dma_gather(
                        g_sb[:, nlo:nlo + nhi, :],
                        xs_allB[:, :gather_elem],
                        idxhi_sb[:, hi_starts[b]:hi_starts[b] + nhi * 8],
                        nhi * 128,
                        nhi * 128,
                        gather_elem,
                        elem_step=C,
                        single_packet=False,
                        queue_num=(2 * b + 1) % nqueues,
                    )
                else:
                    nc.gpsimd.memset(g_sb[:, :1, :], 0.0)
                dinvb_sb = opool.tile([1, BLK], f32, tag="dinvb")
                nd_ = nw * WDST
                nc.sync.dma_start(
                    dinvb_sb[:1, :nd_],
                    dinvrow_hbm[BLK * b:BLK * b + nd_]
                    .rearrange("(o n) -> o n", o=1))
                gtiles[bi] = (g_sb, dinvb_sb)

            issue_block_loads(0)
            issue_s_builds(0)
            if len(blk_list) > 1:
                issue_block_loads(1)
                issue_s_builds(1)
            for bi, b in enumerate(blk_list):
                nw = nw_b[b]
                c0 = blk_start[b]
                nd = nw * WDST
                novf = ovl_b[b] + ovh_b[b]
                g_sb, dinvb_sb = gtiles.pop(bi)
                s_blk, sov = stiles.pop(bi)
                pe_absorb(g_sb[:1, :1, :1])
                if bi + 2 < len(blk_list):
                    issue_block_loads(bi + 2)
                    issue_s_builds(bi + 2)

                base, ovf = block_col_map(b)
                agg_ps = pagg.tile([128, BLK], f32, tag="agg_ps")
                if skip_compute:
                    nc.vector.memset(agg_ps[:, :nd], 0.0)
                if not skip_compute:
                    bywin = {w: [] for w in range(nw)}
                    for bidx, (w, h, j, ca) in enumerate(base):
                        bywin[w].append((ca, s_blk, bidx, None))
                    for w in range(nw):
                        # overflow columns contribute a 128-wide segment to
                        # every window of the block
                        for ca, k in ovf:
                            bywin[w].append((ca, sov, k, w))
                    for w in range(nw):
                        t = WPB * b + w
                        # self-loop contribution: += xs_own[window w]^T
                        nc.tensor.matmul(
                            agg_ps[:, w * WDST:(w + 1) * WDST],
                            lhsT=xs_sb[:, t, :], rhs=eye_sb[:],
                            start=True, stop=False)
                        cols = bywin[w]
                        for ci, (ca, stile, k, seg) in enumerate(cols):
                            rhs = (stile[:, k, :] if seg is None else
                                   stile[:, k, seg * WDST:(seg + 1) * WDST])
                            nc.tensor.matmul(
                                agg_ps[:, w * WDST:(w + 1) * WDST],
                                lhsT=g_sb[:, ca, :],
                                rhs=rhs,
                                start=False,
                                stop=(ci == len(cols) - 1),
                            )

                agg16_sb = bpool.tile([128, BLK], f16, tag="agg16")
                nc.scalar.activation(agg16_sb[:, :nd], agg_ps[:, :nd],
                                     AF.Copy)
                h_ps = ph.tile([128, BLK], f32, tag="h")
                nc.tensor.matmul(h_ps[:, :nd], lhsT=w16_sb[:],
                                 rhs=agg16_sb[:, :nd], start=True, stop=True)
                r_sb = bpool.tile([128, BLK], f16, tag="r")
                nc.scalar.activation(r_sb[:, :nd], h_ps[:, :nd], AF.Relu)
                o_ps = po.tile([1, BLK], f32)
                nc.tensor.matmul(o_ps[:1, :nd], lhsT=linw16_sb[:],
                                 rhs=r_sb[:, :nd], start=True, stop=True)
                orow_sb = opool.tile([1, BLK], f32, tag="orow")
                nc.vector.tensor_tensor(
                    out=orow_sb[:1, :nd], in0=o_ps[:1, :nd],
                    in1=dinvb_sb[:1, :nd], op=OP.mult)
                orow2_sb = opool.tile([1, BLK], f32, tag="orow2")
                nc.vector.tensor_scalar(
                    out=orow2_sb[:1, :nd], in0=orow_sb[:1, :nd],
                    scalar1=linb_sb[:1, :1], scalar2=None, op0=OP.add)
                nc.sync.dma_start(out_t[BLK * b:BLK * b + nd],
                                  orow2_sb[:1, :nd])
    nc.compile()
    return nc


# ---------------------------------------------------------------------------
# Entry point
# ---------------------------------------------------------------------------

_PROG_CACHE = {}


def kernel(x, edge_index, edge_weight, W0, gru_w_ih, gru_w_hh,
           gru_b_ih, gru_b_hh, lin_w, lin_b):
    from concourse.bass_utils import run_bass_kernel_spmd

    pre, metas = preprocess(np.asarray(edge_index), np.asarray(edge_weight))
    key = (pre["K0"], pre["KDEG"], pre["TOT"], tuple(pre["ovl_b"]),
           tuple(pre["ovh_b"]))
    if key not in _PROG_CACHE:
        _PROG_CACHE[key] = build_program(pre)
    nc = _PROG_CACHE[key]
    inp = dict(x=x, W0=W0, gru_w_ih=gru_w_ih, gru_w_hh=gru_w_hh,
               gru_b_ih=gru_b_ih, gru_b_hh=gru_b_hh, lin_w=lin_w, lin_b=lin_b)
    in_maps = make_in_maps(inp, pre, metas)
    res = run_bass_kernel_spmd(nc, in_maps, list(range(NCORES)))
    out = np.concatenate([np.asarray(res.results[c]["out"])[:NPC]
                          for c in range(NCORES)])
    full = np.empty(N_NODES, np.float32)
    full[pre["node_of_slot"]] = out
    return full.reshape(N_NODES, 1).astype(np.float32)
